# revision 9
# baseline (speedup 1.0000x reference)
import numpy as np
import concourse.bass as bass
import concourse.tile as tile
import concourse.mybir as mybir
from concourse.bass_utils import run_bass_kernel_spmd
from concourse.vector_clock import ScopedClock

AF = mybir.ActivationFunctionType
f32 = mybir.dt.float32
f32r = mybir.dt.float32r


# ---- walrus workaround: at most 1 sync wait per engine instruction ----
def _patched_drain_and_barrier(self, tick_clock, wait_clock):
    drain_inst = self.nc.sync.drain()
    wait_clock.add_sem_waits(
        drain_inst.ins, ScopedClock({None: tick_clock.global_clock})
    )
    si = drain_inst.ins.sync_info
    if si is not None and len(si.on_wait) > 1:
        waits = list(si.on_wait)
        drain_inst.ins.sync_info = mybir.SyncInfo(
            on_wait=waits[:1], on_update=list(si.on_update)
        )
        for w in waits[1:]:
            nop = self.nc.sync.nop(nofuse=True)
            nop.ins.sync_info = mybir.SyncInfo(on_wait=[w], on_update=[])
    self.nc.all_engine_barrier()
    assert self.sems is not None
    popped = self.nc._tile_sem_poison_stack.pop()
    assert popped is self._sem_poison
    self.nc.clear_and_free_semaphores(list(self.sems.allocated().values()))
    self.nc.all_engine_barrier()


tile.TileContext._drain_and_barrier = _patched_drain_and_barrier

_orig_commit = tile.TileContext._commit_instruction


def _commit_split_waits(self, inst, lazy_reg_writes=True):
    si = inst.sync_info
    if (
        si is not None
        and len(si.on_wait) > 1
        and inst.engine != mybir.EngineType.Unassigned
    ):
        waits = list(si.on_wait)
        for w in waits[:-1]:
            nop = mybir.InstNoOp(
                name=self.nc.get_next_instruction_name(),
                sync_info=mybir.SyncInfo(on_wait=[w], on_update=[]),
                bass_nofuse=True,
                engine=inst.engine,
            )
            _orig_commit(self, nop, lazy_reg_writes=False)
        inst.sync_info = mybir.SyncInfo(
            on_wait=[waits[-1]], on_update=list(si.on_update)
        )
    _orig_commit(self, inst, lazy_reg_writes)


tile.TileContext._commit_instruction = _commit_split_waits


B, S, E, H, Dh = 4, 8192, 768, 12, 64
N_CORES = 8
SC = S // 2      # tokens per core
NT = SC // 128   # 32 output tiles
NW = SC // 512   # 8 query windows
KC = E // 128    # 6 contraction chunks


def _build_program():
    nc = bass.Bass()
    xk_t = nc.declare_dram_parameter("xk_t", [NT, 128, KC, 128], f32, isOutput=False)
    xv_t = nc.declare_dram_parameter("xv_t", [NT, 128, KC, 128], f32, isOutput=False)
    xq_t = nc.declare_dram_parameter("xq_t", [NW, 128, KC, 512], f32, isOutput=False)
    wk_h = nc.declare_dram_parameter("wk_h", [128, KC, E], f32, isOutput=False)
    wv_h = nc.declare_dram_parameter("wv_h", [128, KC, E], f32, isOutput=False)
    wq_h = nc.declare_dram_parameter("wq_h", [128, KC, E], f32, isOutput=False)
    wo_h = nc.declare_dram_parameter("wo_h", [128, KC, E], f32, isOutput=False)
    bk_h = nc.declare_dram_parameter("bk_h", [1, E], f32, isOutput=False)
    bv_h = nc.declare_dram_parameter("bv_h", [1, E], f32, isOutput=False)
    bq_h = nc.declare_dram_parameter("bq_h", [1, E], f32, isOutput=False)
    bo_h = nc.declare_dram_parameter("bo_h", [1, E], f32, isOutput=False)
    ones_h = nc.declare_dram_parameter("ones_h", [1, 512], f32, isOutput=False)
    y = nc.declare_dram_parameter("y", [NT, 128, E], f32, isOutput=True)

    with tile.TileContext(nc) as tc:
        with (
            tc.tile_pool(name="main", bufs=1) as main,
            tc.tile_pool(name="dram", bufs=1, space="DRAM") as dram,
        ):
            wk_sb = main.tile([128, KC, E], f32r, tag="wk")
            wv_sb = main.tile([128, KC, E], f32r, tag="wv")
            bk_sb = main.tile([1, E], f32r, tag="bk")
            bv_sb = main.tile([1, E], f32r, tag="bv")
            ones_sb = main.tile([1, 512], f32r, tag="ones")
            nc.sync.dma_start(out=wk_sb[:], in_=wk_h[:].bitcast(f32r))
            nc.sync.dma_start(out=wv_sb[:], in_=wv_h[:].bitcast(f32r))
            nc.sync.dma_start(out=bk_sb[:], in_=bk_h[:].bitcast(f32r))
            nc.sync.dma_start(out=bv_sb[:], in_=bv_h[:].bitcast(f32r))
            nc.sync.dma_start(out=ones_sb[:], in_=ones_h[:].bitcast(f32r))

            wq_sb = main.tile([128, KC, E], f32r, tag="wq")
            wo_sb = main.tile([128, KC, E], f32r, tag="wo")
            bq_sb = main.tile([1, E], f32r, tag="bq")
            bo_sb = main.tile([1, E], f32r, tag="bo")

            kvc = main.tile([128, KC, 128], f32, tag="kvc")
            kvbd = main.tile([128, KC, 128], f32r, tag="kvbd")
            kv_in = dram.tile([128, KC, 128], f32, tag="kvin")
            kv_out = dram.tile([128, KC, 128], f32, tag="kvout")

            # ---------------- phase 1: K/V projection + KV accumulation ----
            with (
                tc.tile_pool(name="p1", bufs=1) as p1,
                tc.tile_pool(name="pp1", bufs=1, space="PSUM") as pp1,
            ):
                kvp = [
                    pp1.tile([128, 256], f32, tag=f"kv{c}", name=f"kvp{c}")
                    for c in range(KC)
                ]
                for t in range(NT):
                    xk_sb = p1.tile([128, KC, 128], f32r, tag="xk", bufs=3,
                                    name=f"xk{t}")
                    xv_sb = p1.tile([128, KC, 128], f32r, tag="xv", bufs=3,
                                    name=f"xv{t}")
                    nc.sync.dma_start(out=xk_sb[:], in_=xk_t[t, :, :, :].bitcast(f32r))
                    nc.sync.dma_start(out=xv_sb[:], in_=xv_t[t, :, :, :].bitcast(f32r))
                    k_sb = p1.tile([128, E], f32r, tag="k", bufs=2, name=f"k{t}")
                    v_sb = p1.tile([128, E], f32r, tag="v", bufs=2, name=f"v{t}")
                    for hh in range(2):
                        psk = pp1.tile([128, 384], f32, tag="pp", bufs=2,
                                       name=f"psk{t}_{hh}")
                        for kb in range(KC):
                            nc.tensor.matmul(
                                psk[:], xk_sb[:, kb, :],
                                wk_sb[:, kb, hh * 384:(hh + 1) * 384],
                                start=(kb == 0), stop=False)
                        nc.tensor.matmul(psk[:], ones_sb[:, 0:128],
                                         bk_sb[:, hh * 384:(hh + 1) * 384],
                                         start=False, stop=True)
                        nc.scalar.activation(
                            k_sb[:, hh * 384:(hh + 1) * 384], psk[:], AF.Relu)
                    for hh in range(2):
                        psv = pp1.tile([128, 384], f32, tag="pp", bufs=2,
                                       name=f"psv{t}_{hh}")
                        for kb in range(KC):
                            nc.tensor.matmul(
                                psv[:], xv_sb[:, kb, :],
                                wv_sb[:, kb, hh * 384:(hh + 1) * 384],
                                start=(kb == 0), stop=False)
                        nc.tensor.matmul(psv[:], ones_sb[:, 0:128],
                                         bv_sb[:, hh * 384:(hh + 1) * 384],
                                         start=False, stop=True)
                        nc.vector.tensor_copy(
                            v_sb[:, hh * 384:(hh + 1) * 384], psv[:])
                    for c in range(KC):
                        w0 = min(128 * c, 512)
                        nc.tensor.matmul(
                            kvp[c][:, :],
                            k_sb[:, c * 128:(c + 1) * 128],
                            v_sb[:, w0:w0 + 256],
                            start=(t == 0), stop=(t == NT - 1))

                # load phase-2 weights during phase-1 compute
                nc.sync.dma_start(out=wq_sb[:], in_=wq_h[:].bitcast(f32r))
                nc.sync.dma_start(out=wo_sb[:], in_=wo_h[:].bitcast(f32r))
                nc.sync.dma_start(out=bq_sb[:], in_=bq_h[:].bitcast(f32r))
                nc.sync.dma_start(out=bo_sb[:], in_=bo_h[:].bitcast(f32r))

                # extract block-diagonal KV pairs into zeroed kvc
                nc.vector.memset(kvc[:], 0.0)
                for c in range(KC):
                    w0 = min(128 * c, 512)
                    o = 128 * c - w0
                    nc.scalar.activation(
                        kvc[0:64, c, 0:64],
                        kvp[c][0:64, o:o + 64], AF.Copy)
                    nc.scalar.activation(
                        kvc[64:128, c, 64:128],
                        kvp[c][64:128, o + 64:o + 128], AF.Copy)

            # ---------------- AllReduce KV over S-halves --------------------
            nc.gpsimd.dma_start(kv_in[:], kvc[:])
            nc.gpsimd.collective_compute(
                "AllReduce",
                mybir.AluOpType.add,
                replica_groups=[[0, 1], [2, 3], [4, 5], [6, 7]],
                ins=[kv_in.opt()],
                outs=[kv_out.opt()],
            )
            nc.gpsimd.dma_start(kvbd[:], kv_out[:].bitcast(f32r))

            # ---------------- phase 2/3: Q proj, QKV, out-proj --------------
            with (
                tc.tile_pool(name="p2", bufs=1) as p2,
                tc.tile_pool(name="pp2", bufs=1, space="PSUM") as pp2,
            ):
                def qproj(n):
                    xq_sb = p2.tile([128, KC, 512], f32r, tag="xq", bufs=2,
                                    name=f"xq{n}")
                    nc.sync.dma_start(out=xq_sb[:],
                                      in_=xq_t[n, :, :, :].bitcast(f32r))
                    qt = p2.tile([128, KC, 512], f32r, tag="qt", bufs=3,
                                 name=f"qt{n}")
                    for m in range(KC):
                        psq = pp2.tile([128, 512], f32, tag="pq", bufs=2,
                                       name=f"psq{n}_{m}")
                        for kb in range(KC):
                            nc.tensor.matmul(
                                psq[:], wq_sb[:, kb, m * 128:(m + 1) * 128],
                                xq_sb[:, kb, :],
                                start=(kb == 0), stop=False)
                        nc.tensor.matmul(psq[:], bq_sb[:, m * 128:(m + 1) * 128],
                                         ones_sb[:], start=False, stop=True)
                        nc.scalar.activation(qt[:, m, :], psq[:], AF.Relu)
                    return qt

                qts = {0: qproj(0)}
                for n in range(NW):
                    if n + 1 < NW:
                        qts[n + 1] = qproj(n + 1)
                    qt = qts.pop(n)
                    qkvt = p2.tile([128, KC, 512], f32r, tag="qkvt", bufs=2,
                                   name=f"qkvt{n}")
                    for c in range(KC):
                        psa = pp2.tile([128, 512], f32, tag="pa", bufs=2,
                                       name=f"psa{n}_{c}")
                        nc.tensor.matmul(psa[:], kvbd[:, c, :], qt[:, c, :],
                                         start=True, stop=True)
                        nc.vector.tensor_copy(qkvt[:, c, :], psa[:])
                    for sub in range(4):
                        o_sb = p2.tile([128, E], f32, tag="o", bufs=2,
                                       name=f"o{n}_{sub}")
                        for nh in range(2):
                            pso = pp2.tile([128, 384], f32, tag="po", bufs=2,
                                           name=f"pso{n}_{sub}_{nh}")
                            for c in range(KC):
                                nc.tensor.matmul(
                                    pso[:],
                                    qkvt[:, c, sub * 128:(sub + 1) * 128],
                                    wo_sb[:, c, nh * 384:(nh + 1) * 384],
                                    start=(c == 0), stop=False)
                            nc.tensor.matmul(pso[:], ones_sb[:, 0:128],
                                             bo_sb[:, nh * 384:(nh + 1) * 384],
                                             start=False, stop=True)
                            if nh == 0:
                                nc.scalar.activation(o_sb[:, 0:384], pso[:],
                                                     AF.Copy)
                            else:
                                nc.vector.tensor_copy(o_sb[:, 384:768], pso[:])
                        nc.sync.dma_start(out=y[n * 4 + sub, :, :], in_=o_sb[:])
    return nc


def _prep_x_tiles(x, tile_free):
    # x: [SC, E] -> [SC//tile_free, 128, KC, tile_free]
    n = SC // tile_free
    return np.ascontiguousarray(
        x.T.reshape(KC, 128, n, tile_free).transpose(2, 1, 0, 3))


def _prep_w(W):
    # W: [E, E] (out, in) -> [128, KC, E] with [p, kb, eo] = W[eo, kb*128+p]
    return np.ascontiguousarray(W.T.reshape(KC, 128, E).transpose(1, 0, 2))


def _make_in_maps(query, key, value, Wq, bq, Wk, bk, Wv, bv, Wo, bo):
    query = np.asarray(query, dtype=np.float32)
    key = np.asarray(key, dtype=np.float32)
    value = np.asarray(value, dtype=np.float32)

    shared = {
        "wq_h": _prep_w(np.asarray(Wq, dtype=np.float32)),
        "wk_h": _prep_w(np.asarray(Wk, dtype=np.float32)),
        "wv_h": _prep_w(np.asarray(Wv, dtype=np.float32)),
        "wo_h": _prep_w(np.asarray(Wo, dtype=np.float32)),
        "bq_h": np.asarray(bq, dtype=np.float32).reshape(1, E),
        "bk_h": np.asarray(bk, dtype=np.float32).reshape(1, E),
        "bv_h": np.asarray(bv, dtype=np.float32).reshape(1, E),
        "bo_h": np.asarray(bo, dtype=np.float32).reshape(1, E),
        "ones_h": np.ones((1, 512), dtype=np.float32),
    }

    in_maps = []
    for i in range(N_CORES):
        b, hf = i // 2, i % 2
        sl = slice(hf * SC, (hf + 1) * SC)
        m = dict(shared)
        m["xq_t"] = _prep_x_tiles(query[b, sl], 512)
        m["xk_t"] = _prep_x_tiles(key[b, sl], 128)
        m["xv_t"] = _prep_x_tiles(value[b, sl], 128)
        in_maps.append(m)
    return in_maps


def kernel(**inputs):
    in_maps = _make_in_maps(**inputs)
    nc = _build_program()
    res = run_bass_kernel_spmd(nc, in_maps, core_ids=list(range(N_CORES)))

    out = np.empty((B, S, E), dtype=np.float32)
    for i in range(N_CORES):
        b, hf = i // 2, i % 2
        out[b, hf * SC:(hf + 1) * SC] = res.results[i]["y"].reshape(SC, E)
    return out


# revision 13
# speedup vs baseline: 1.2284x; 1.2284x over previous
import numpy as np
import concourse.bass as bass
import concourse.tile as tile
import concourse.mybir as mybir
from concourse.bass_utils import run_bass_kernel_spmd
from concourse.vector_clock import ScopedClock

AF = mybir.ActivationFunctionType
f32 = mybir.dt.float32
f32r = mybir.dt.float32r


# ---- walrus workaround: at most 1 sync wait per engine instruction ----
def _patched_drain_and_barrier(self, tick_clock, wait_clock):
    drain_inst = self.nc.sync.drain()
    wait_clock.add_sem_waits(
        drain_inst.ins, ScopedClock({None: tick_clock.global_clock})
    )
    si = drain_inst.ins.sync_info
    if si is not None and len(si.on_wait) > 1:
        waits = list(si.on_wait)
        drain_inst.ins.sync_info = mybir.SyncInfo(
            on_wait=waits[:1], on_update=list(si.on_update)
        )
        for w in waits[1:]:
            nop = self.nc.sync.nop(nofuse=True)
            nop.ins.sync_info = mybir.SyncInfo(on_wait=[w], on_update=[])
    self.nc.all_engine_barrier()
    assert self.sems is not None
    popped = self.nc._tile_sem_poison_stack.pop()
    assert popped is self._sem_poison
    self.nc.clear_and_free_semaphores(list(self.sems.allocated().values()))
    self.nc.all_engine_barrier()


tile.TileContext._drain_and_barrier = _patched_drain_and_barrier

_orig_commit = tile.TileContext._commit_instruction


def _commit_split_waits(self, inst, lazy_reg_writes=True):
    si = inst.sync_info
    if (
        si is not None
        and len(si.on_wait) > 1
        and inst.engine != mybir.EngineType.Unassigned
    ):
        waits = list(si.on_wait)
        for w in waits[:-1]:
            nop = mybir.InstNoOp(
                name=self.nc.get_next_instruction_name(),
                sync_info=mybir.SyncInfo(on_wait=[w], on_update=[]),
                bass_nofuse=True,
                engine=inst.engine,
            )
            _orig_commit(self, nop, lazy_reg_writes=False)
        inst.sync_info = mybir.SyncInfo(
            on_wait=[waits[-1]], on_update=list(si.on_update)
        )
    _orig_commit(self, inst, lazy_reg_writes)


tile.TileContext._commit_instruction = _commit_split_waits


B, S, E, H, Dh = 4, 8192, 768, 12, 64
N_CORES = 8
SC = S // 2      # tokens per core
NT = SC // 128   # 32 output tiles
NW = SC // 512   # 8 query windows
KC = E // 128    # 6 contraction chunks


def _build_program():
    nc = bass.Bass()
    xk_t = nc.declare_dram_parameter("xk_t", [NT, 128, KC, 128], f32, isOutput=False)
    xv_t = nc.declare_dram_parameter("xv_t", [NT, 128, KC, 128], f32, isOutput=False)
    xq_t = nc.declare_dram_parameter("xq_t", [NW, 128, KC, 512], f32, isOutput=False)
    wk_h = nc.declare_dram_parameter("wk_h", [128, KC, E], f32, isOutput=False)
    wv_h = nc.declare_dram_parameter("wv_h", [128, KC, E], f32, isOutput=False)
    wq_h = nc.declare_dram_parameter("wq_h", [128, KC, E], f32, isOutput=False)
    wo_h = nc.declare_dram_parameter("wo_h", [128, KC, E], f32, isOutput=False)
    bkb_h = nc.declare_dram_parameter("bkb_h", [128, E], f32, isOutput=False)
    bvb_h = nc.declare_dram_parameter("bvb_h", [128, E], f32, isOutput=False)
    bqc_h = nc.declare_dram_parameter("bqc_h", [128, KC], f32, isOutput=False)
    boc_h = nc.declare_dram_parameter("boc_h", [128, KC], f32, isOutput=False)
    y_t = nc.declare_dram_parameter("y_t", [NW, 128, KC, 512], f32, isOutput=True)

    with tile.TileContext(nc) as tc:
        with (
            tc.tile_pool(name="main", bufs=1) as main,
            tc.tile_pool(name="dram", bufs=1, space="DRAM") as dram,
        ):
            wq_sb = main.tile([128, KC, E], f32r, tag="wq")
            wo_sb = main.tile([128, KC, E], f32r, tag="wo")
            bkb_sb = main.tile([128, E], f32, tag="bkb")
            bvb_sb = main.tile([128, E], f32, tag="bvb")
            bqc_sb = main.tile([128, KC], f32, tag="bqc")
            boc_sb = main.tile([128, KC], f32, tag="boc")

            kvc = main.tile([128, KC, 128], f32, tag="kvc")
            kvbd = main.tile([128, KC, 128], f32r, tag="kvbd")
            kv_in = dram.tile([128, KC, 128], f32, tag="kvin")
            kv_out = dram.tile([128, KC, 128], f32, tag="kvout")

            # ---------------- phase 1: K/V projection + KV accumulation ----
            with (
                tc.tile_pool(name="p1", bufs=1) as p1,
                tc.tile_pool(name="pp1", bufs=1, space="PSUM") as pp1,
            ):
                wk_sb = p1.tile([128, KC, E], f32r, tag="wk")
                wv_sb = p1.tile([128, KC, E], f32r, tag="wv")
                # weight/bias loads on the gpsimd DMA queue so they do not
                # serialize behind the per-tile x DMAs on the sync queue
                nc.gpsimd.dma_start(wk_sb[:], wk_h[:].bitcast(f32r))
                nc.gpsimd.dma_start(wv_sb[:], wv_h[:].bitcast(f32r))
                nc.gpsimd.dma_start(bkb_sb[:], bkb_h[:])
                nc.gpsimd.dma_start(bvb_sb[:], bvb_h[:])
                nc.gpsimd.dma_start(bqc_sb[:], bqc_h[:])
                nc.gpsimd.dma_start(boc_sb[:], boc_h[:])
                nc.gpsimd.dma_start(wq_sb[:], wq_h[:].bitcast(f32r))
                nc.gpsimd.dma_start(wo_sb[:], wo_h[:].bitcast(f32r))

                kvp = [
                    pp1.tile([128, 128], f32, tag=f"kv{c}", name=f"kvp{c}")
                    for c in range(KC)
                ]
                for t in range(NT):
                    xk_sb = p1.tile([128, KC, 128], f32r, tag="xk", bufs=3,
                                    name=f"xk{t}")
                    xv_sb = p1.tile([128, KC, 128], f32r, tag="xv", bufs=3,
                                    name=f"xv{t}")
                    nc.sync.dma_start(out=xk_sb[:], in_=xk_t[t, :, :, :].bitcast(f32r))
                    nc.sync.dma_start(out=xv_sb[:], in_=xv_t[t, :, :, :].bitcast(f32r))
                    k_sb = p1.tile([128, E], f32r, tag="k", bufs=2, name=f"k{t}")
                    v_sb = p1.tile([128, E], f32r, tag="v", bufs=2, name=f"v{t}")
                    for hh in range(2):
                        psk = pp1.tile([128, 384], f32, tag="pp", bufs=2,
                                       name=f"psk{t}_{hh}")
                        # preload bias into psum, then accumulate on top
                        nc.scalar.activation(
                            psk[:], bkb_sb[:, hh * 384:(hh + 1) * 384], AF.Copy)
                        for kb in range(KC):
                            nc.tensor.matmul(
                                psk[:], xk_sb[:, kb, :],
                                wk_sb[:, kb, hh * 384:(hh + 1) * 384],
                                start=False, stop=(kb == KC - 1),
                                skip_group_check=True)
                        nc.scalar.activation(
                            k_sb[:, hh * 384:(hh + 1) * 384], psk[:], AF.Relu)
                    for hh in range(2):
                        psv = pp1.tile([128, 384], f32, tag="pp", bufs=2,
                                       name=f"psv{t}_{hh}")
                        nc.scalar.activation(
                            psv[:], bvb_sb[:, hh * 384:(hh + 1) * 384], AF.Copy)
                        for kb in range(KC):
                            nc.tensor.matmul(
                                psv[:], xv_sb[:, kb, :],
                                wv_sb[:, kb, hh * 384:(hh + 1) * 384],
                                start=False, stop=(kb == KC - 1),
                                skip_group_check=True)
                        nc.vector.tensor_copy(
                            v_sb[:, hh * 384:(hh + 1) * 384], psv[:])
                    for c in range(KC):
                        nc.tensor.matmul(
                            kvp[c][:, :],
                            k_sb[:, c * 128:(c + 1) * 128],
                            v_sb[:, c * 128:(c + 1) * 128],
                            start=(t == 0), stop=(t == NT - 1))

                # extract block-diagonal KV pairs into zeroed kvc
                nc.vector.memset(kvc[:], 0.0)
                for c in range(KC):
                    nc.scalar.activation(
                        kvc[0:64, c, 0:64],
                        kvp[c][0:64, 0:64], AF.Copy)
                    nc.scalar.activation(
                        kvc[64:128, c, 64:128],
                        kvp[c][64:128, 64:128], AF.Copy)

            # ---------------- AllReduce KV over S-halves --------------------
            nc.gpsimd.dma_start(kv_in[:], kvc[:])
            nc.gpsimd.collective_compute(
                "AllReduce",
                mybir.AluOpType.add,
                replica_groups=[[0, 1], [2, 3], [4, 5], [6, 7]],
                ins=[kv_in.opt()],
                outs=[kv_out.opt()],
            )
            nc.gpsimd.dma_start(kvbd[:], kv_out[:].bitcast(f32r))

            # ---------------- phase 2/3: Q proj, QKV, out-proj --------------
            with (
                tc.tile_pool(name="p2", bufs=1) as p2,
                tc.tile_pool(name="pp2", bufs=1, space="PSUM") as pp2,
            ):
                def qproj(n):
                    xq_sb = p2.tile([128, KC, 512], f32r, tag="xq", bufs=2,
                                    name=f"xq{n}")
                    nc.sync.dma_start(out=xq_sb[:],
                                      in_=xq_t[n, :, :, :].bitcast(f32r))
                    qt = p2.tile([128, KC, 512], f32r, tag="qt", bufs=3,
                                 name=f"qt{n}")
                    for m in range(KC):
                        psq = pp2.tile([128, 512], f32, tag="pq", bufs=2,
                                       name=f"psq{n}_{m}")
                        for kb in range(KC):
                            nc.tensor.matmul(
                                psq[:], wq_sb[:, kb, m * 128:(m + 1) * 128],
                                xq_sb[:, kb, :],
                                start=(kb == 0), stop=(kb == KC - 1))
                        nc.scalar.activation(qt[:, m, :], psq[:], AF.Relu,
                                             bias=bqc_sb[:, m:m + 1])
                    return qt

                qts = {0: qproj(0)}
                for n in range(NW):
                    if n + 1 < NW:
                        qts[n + 1] = qproj(n + 1)
                    qt = qts.pop(n)
                    qkvt = p2.tile([128, KC, 512], f32r, tag="qkvt", bufs=2,
                                   name=f"qkvt{n}")
                    for c in range(KC):
                        psa = pp2.tile([128, 512], f32, tag="pa", bufs=2,
                                       name=f"psa{n}_{c}")
                        nc.tensor.matmul(psa[:], kvbd[:, c, :], qt[:, c, :],
                                         start=True, stop=True)
                        nc.vector.tensor_copy(qkvt[:, c, :], psa[:])
                    yt_sb = p2.tile([128, KC, 512], f32, tag="yt", bufs=2,
                                    name=f"yt{n}")
                    for m in range(KC):
                        pso = pp2.tile([128, 512], f32, tag="po", bufs=2,
                                       name=f"pso{n}_{m}")
                        for c in range(KC):
                            nc.tensor.matmul(
                                pso[:],
                                wo_sb[:, c, m * 128:(m + 1) * 128],
                                qkvt[:, c, :],
                                start=(c == 0), stop=(c == KC - 1))
                        nc.scalar.activation(yt_sb[:, m, :], pso[:],
                                             AF.Identity,
                                             bias=boc_sb[:, m:m + 1])
                    nc.sync.dma_start(out=y_t[n, :, :, :], in_=yt_sb[:])
    return nc


def _prep_x_tiles(x, tile_free):
    # x: [SC, E] -> [SC//tile_free, 128, KC, tile_free]
    n = SC // tile_free
    return np.ascontiguousarray(
        x.T.reshape(KC, 128, n, tile_free).transpose(2, 1, 0, 3))


def _prep_w(W):
    # W: [E, E] (out, in) -> [128, KC, E] with [p, kb, eo] = W[eo, kb*128+p]
    return np.ascontiguousarray(W.T.reshape(KC, 128, E).transpose(1, 0, 2))


def _make_in_maps(query, key, value, Wq, bq, Wk, bk, Wv, bv, Wo, bo):
    query = np.asarray(query, dtype=np.float32)
    key = np.asarray(key, dtype=np.float32)
    value = np.asarray(value, dtype=np.float32)

    shared = {
        "wq_h": _prep_w(np.asarray(Wq, dtype=np.float32)),
        "wk_h": _prep_w(np.asarray(Wk, dtype=np.float32)),
        "wv_h": _prep_w(np.asarray(Wv, dtype=np.float32)),
        "wo_h": _prep_w(np.asarray(Wo, dtype=np.float32)),
        "bkb_h": np.ascontiguousarray(
            np.broadcast_to(np.asarray(bk, dtype=np.float32), (128, E))),
        "bvb_h": np.ascontiguousarray(
            np.broadcast_to(np.asarray(bv, dtype=np.float32), (128, E))),
        "bqc_h": np.ascontiguousarray(
            np.asarray(bq, dtype=np.float32).reshape(KC, 128).T),
        "boc_h": np.ascontiguousarray(
            np.asarray(bo, dtype=np.float32).reshape(KC, 128).T),
    }

    in_maps = []
    for i in range(N_CORES):
        b, hf = i // 2, i % 2
        sl = slice(hf * SC, (hf + 1) * SC)
        m = dict(shared)
        m["xq_t"] = _prep_x_tiles(query[b, sl], 512)
        m["xk_t"] = _prep_x_tiles(key[b, sl], 128)
        m["xv_t"] = _prep_x_tiles(value[b, sl], 128)
        in_maps.append(m)
    return in_maps


def kernel(**inputs):
    in_maps = _make_in_maps(**inputs)
    nc = _build_program()
    res = run_bass_kernel_spmd(nc, in_maps, core_ids=list(range(N_CORES)))

    out = np.empty((B, S, E), dtype=np.float32)
    for i in range(N_CORES):
        b, hf = i // 2, i % 2
        yt = res.results[i]["y_t"]  # [NW, 128, KC, 512]
        out[b, hf * SC:(hf + 1) * SC] = (
            yt.transpose(0, 3, 2, 1).reshape(SC, E))
    return out
